# revision 20
# baseline (speedup 1.0000x reference)
"""CLUB loss kernel for Trainium2 (8 NeuronCores, SPMD row-sharded).

Math: the reference returns mean_i(pos_i - neg_i), a scalar.  Both the
pos and neg terms collapse into sums that never materialize the NxN
distance matrix:

  mean_pos = -0.5/N * (A - 2B + C)
      A = sum_{i,d} x[i,d]^2 * invv[i,d]
      B = sum_{i,d} x[i,d] * mu[i,d] * invv[i,d]
      C = sum_{i,d} mu[i,d]^2 * invv[i,d]
  mean_neg = -0.5 * (S_invv . S_x2 - 2 * S_muinvv . S_x + N*C) / N^2
      S_invv = sum_i invv[i,:]     S_muinvv = sum_i mu[i,:]*invv[i,:]
      S_x    = sum_j x[j,:]        S_x2     = sum_j x[j,:]^2
  loss = mean_pos - mean_neg

Each core handles 2048 rows (2 batches of x + matching mu/logvar rows)
and emits f32 partial sums; the host combines them in float64.
"""

import sys

sys.path.insert(0, "/opt/trn_rl_repo")

import numpy as np
from contextlib import ExitStack

import concourse.bass as bass
import concourse.bacc as bacc
import concourse.tile as tile
from concourse import mybir
from concourse.masks import make_identity
from concourse.bass_utils import run_bass_kernel_spmd

F32 = mybir.dt.float32
N_CORES = 8
B, D, H, W = 16, 64, 32, 32
HW = H * W                # 1024
N = B * HW                # 16384
NB = B // N_CORES         # 2 batches per core
ROWS = NB * HW            # 2048 rows per core
NT = ROWS // 128          # 16 tiles of 128 rows per core
CH = 4                    # mu/logvar DMA chunks (1KB contiguous per partition)


def build_nc() -> bass.Bass:
    nc = bacc.Bacc()
    xs = nc.dram_tensor("xs", [NB, D, HW], F32, kind="ExternalInput")
    mus = nc.dram_tensor("mus", [ROWS, D], F32, kind="ExternalInput")
    lvs = nc.dram_tensor("lvs", [ROWS, D], F32, kind="ExternalInput")
    # accs columns: [A partials | B partials | C partials], NT each
    accs = nc.dram_tensor("accs", [128, 3 * NT], F32, kind="ExternalOutput")
    # vrow: [S_invv | S_x | S_x2 | S_muinvv], 64 each
    vrow = nc.dram_tensor("vrow", [1, 4 * D], F32, kind="ExternalOutput")

    with ExitStack() as ctx:
        tc = ctx.enter_context(tile.TileContext(nc))
        consts = ctx.enter_context(tc.tile_pool(name="consts", bufs=1))
        big = ctx.enter_context(tc.tile_pool(name="big", bufs=1))
        # bufs=NT: every tile gets a fresh slot, so no cross-engine WAR/WAW
        # waits from slot reuse (HW engine instructions only support a small
        # number of sync-wait commands; PE supports just one).
        work = ctx.enter_context(tc.tile_pool(name="work", bufs=NT))
        accp = ctx.enter_context(tc.tile_pool(name="accp", bufs=1))
        pt = ctx.enter_context(tc.tile_pool(name="pt", bufs=4, space="PSUM"))
        pacc = ctx.enter_context(tc.tile_pool(name="pacc", bufs=1, space="PSUM"))

        ident = consts.tile([64, 64], F32)
        make_identity(nc, ident)
        # PE matmuls only support a single sync-wait in codegen.  This
        # warm-up transpose (dead output) absorbs the wait on the
        # gpsimd-built identity into an early PE tick so the first real
        # transpose only needs its DMA wait.
        warm = ctx.enter_context(tc.tile_pool(name="warm", bufs=1, space="PSUM"))
        warm_ps = warm.tile([64, 64], F32)
        nc.tensor.transpose(warm_ps[:], ident[:], ident[:])
        # zerob/ones on the scalar engine: zerob is the bias AP of every
        # ACT activation and ones is read by PE matmuls; ACT/PE
        # instructions only support one sync wait, so producing these on
        # ACT lets that wait be subsumed by the ACT data dependency.
        zerob = consts.tile([128, 1], F32)
        nc.scalar.memzero(zerob[:])
        ones = consts.tile([128, 1], F32)
        nc.scalar.add(ones[:], zerob[:], 1.0)

        # x: d on partitions 0..63, free axis = local row il = bl*HW + hw,
        # so column il of xball is exactly flat_x row il (transposed).
        # Single dma_start so downstream transposes wait on one queue sem.
        xball = big.tile([64, NB * HW], F32)
        xs_ap = xs[:, :, :]
        xs_t = bass.AP(
            tensor=xs_ap.tensor,
            offset=xs_ap.offset,
            ap=[[HW, D], [D * HW, NB], [1, HW]],  # d | b, h
        )
        nc.sync.dma_start(
            out=xball[:, :].rearrange("p (b h) -> p b h", b=NB), in_=xs_t
        )

        # mu/logvar: partition p holds rows [16p, 16p+16) contiguously, so the
        # DMA reads 4KB-contiguous per partition (line rate).  Tile n (of 16)
        # is the column block [n*64, n*64+64) = row 16p+n on partition p.
        muall = big.tile([128, NT * D], F32)
        lvall = big.tile([128, NT * D], F32)
        mus_r = mus[:, :].rearrange("(p k) d -> p (k d)", p=128)
        lvs_r = lvs[:, :].rearrange("(p k) d -> p (k d)", p=128)
        chw = NT // CH * D
        for q in range(CH):
            nc.sync.dma_start(
                out=muall[:, q * chw:(q + 1) * chw],
                in_=mus_r[:, q * chw:(q + 1) * chw],
            )
            nc.sync.dma_start(
                out=lvall[:, q * chw:(q + 1) * chw],
                in_=lvs_r[:, q * chw:(q + 1) * chw],
            )

        acc = accp.tile([128, 3 * NT], F32)
        psum_s = pacc.tile([1, 3 * D], F32)   # [S_invv | S_x | S_x2]
        psum_v = pacc.tile([1, D], F32)       # S_muinvv

        # Strided view of x: tile n needs rows il = 16p+n for p = 0..127,
        # which is columns n::16 of xball -> one (64,128) transpose input.
        xst = xball[:, :].rearrange("p (h s) -> p s h", s=16)

        touchp = ctx.enter_context(tc.tile_pool(name="touchp", bufs=CH))

        for n in range(NT):
            mu_t = muall[:, n * D:(n + 1) * D]
            lv_t = lvall[:, n * D:(n + 1) * D]

            if n % (NT // CH) == 0:
                # DVE instructions support one sync wait; mu_invv below
                # needs both the ACT (invv) and DMA (mu chunk) sems at
                # chunk boundaries.  This throwaway copy makes DVE observe
                # the mu-chunk DMA sem first.
                touch = touchp.tile([128, 1], F32, tag="touch")
                nc.vector.tensor_copy(touch[:], muall[:, n * D:n * D + 1])

            xt_ps = pt.tile([128, D], F32)
            nc.tensor.transpose(xt_ps[:], xst[:, n, :], ident[:])

            sc3 = work.tile([128, 3 * D], F32, tag="sc3")
            invv = sc3[:, 0:D]
            xt = sc3[:, D:2 * D]
            x2t = sc3[:, 2 * D:3 * D]
            nc.scalar.activation(
                out=invv, in_=lv_t, func=mybir.ActivationFunctionType.Exp,
                bias=zerob[:], scale=-1.0,
            )
            nc.scalar.copy(out=xt, in_=xt_ps[:])
            nc.scalar.activation(
                out=x2t, in_=xt_ps[:], func=mybir.ActivationFunctionType.Square,
                bias=zerob[:], scale=1.0,
            )

            mu_invv = work.tile([128, D], F32, tag="mi")
            nc.vector.tensor_mul(mu_invv[:], mu_t, invv)

            scr = work.tile([128, D], F32, tag="scr")
            for col, in0, in1 in (
                (n, x2t, invv),              # A partial
                (NT + n, xt, mu_invv[:]),    # B partial
                (2 * NT + n, mu_t, mu_invv[:]),  # C partial
            ):
                # out = in0 * in1, accum_out = row-sum(out): one fused DVE op
                # (tensor_tensor_reduce is not supported by this HW runtime)
                nc.vector.scalar_tensor_tensor(
                    out=scr[:], in0=in0, scalar=1.0, in1=in1,
                    op0=mybir.AluOpType.mult, op1=mybir.AluOpType.mult,
                    accum_out=acc[:, col:col + 1],
                )

            nc.tensor.matmul(
                psum_s[:], ones[:], sc3[:], start=(n == 0), stop=(n == NT - 1)
            )
            nc.tensor.matmul(
                psum_v[:], ones[:], mu_invv[:], start=(n == 0), stop=(n == NT - 1)
            )

        vtile = accp.tile([1, 4 * D], F32)
        nc.scalar.copy(out=vtile[0:1, 0:3 * D], in_=psum_s[:])
        nc.scalar.copy(out=vtile[0:1, 3 * D:4 * D], in_=psum_v[:])
        nc.sync.dma_start(out=accs[:, :], in_=acc[:])
        nc.sync.dma_start(out=vrow[:, :], in_=vtile[:])
    return nc


_NC = None


def _get_nc():
    global _NC
    if _NC is None:
        _NC = build_nc()
        # bacc passes (move_matmul_waits_to_ldweights,
        # generate_event_semaphores, ...) legalize multi-sync-wait
        # instructions for TRN2 codegen.
        _NC.compile()
    return _NC


def make_in_maps(x, mu, logvar):
    x = np.ascontiguousarray(np.asarray(x, dtype=np.float32))
    mu = np.asarray(mu, dtype=np.float32)
    lv = np.asarray(logvar, dtype=np.float32)
    in_maps = []
    for c in range(N_CORES):
        in_maps.append({
            "xs": x[c * NB:(c + 1) * NB].reshape(NB, D, HW),
            "mus": np.ascontiguousarray(mu[c * ROWS:(c + 1) * ROWS]),
            "lvs": np.ascontiguousarray(lv[c * ROWS:(c + 1) * ROWS]),
        })
    return in_maps


def combine(results) -> np.ndarray:
    A = 0.0
    Bs = 0.0
    C = 0.0
    V = np.zeros((4, D), dtype=np.float64)
    for r in results:
        a = np.asarray(r["accs"], dtype=np.float64)
        A += a[:, 0:NT].sum()
        Bs += a[:, NT:2 * NT].sum()
        C += a[:, 2 * NT:3 * NT].sum()
        v = np.asarray(r["vrow"], dtype=np.float64).reshape(4 * D)
        V[0] += v[0:D]          # S_invv
        V[1] += v[D:2 * D]      # S_x
        V[2] += v[2 * D:3 * D]  # S_x2
        V[3] += v[3 * D:4 * D]  # S_muinvv
    mean_pos = -0.5 / N * (A - 2.0 * Bs + C)
    mean_D = (V[0] @ V[2] - 2.0 * V[3] @ V[1] + N * C) / float(N) ** 2
    loss = mean_pos + 0.5 * mean_D
    return np.array(loss, dtype=np.float32)


def kernel(x, mu, logvar, **_kwargs):
    nc = _get_nc()
    in_maps = make_in_maps(x, mu, logvar)
    res = run_bass_kernel_spmd(nc, in_maps, list(range(N_CORES)))
    return combine(res.results)


# revision 22
# speedup vs baseline: 1.1700x; 1.1700x over previous
"""CLUB loss kernel for Trainium2 (8 NeuronCores, SPMD row-sharded).

Math: the reference returns mean_i(pos_i - neg_i), a scalar.  Both the
pos and neg terms collapse into sums that never materialize the NxN
distance matrix:

  mean_pos = -0.5/N * (A - 2B + C)
      A = sum_{i,d} x[i,d]^2 * invv[i,d]
      B = sum_{i,d} x[i,d] * mu[i,d] * invv[i,d]
      C = sum_{i,d} mu[i,d]^2 * invv[i,d]
  mean_neg = -0.5 * (S_invv . S_x2 - 2 * S_muinvv . S_x + N*C) / N^2
      S_invv = sum_i invv[i,:]     S_muinvv = sum_i mu[i,:]*invv[i,:]
      S_x    = sum_j x[j,:]        S_x2     = sum_j x[j,:]^2
  loss = mean_pos - mean_neg

Each core handles 2048 rows (2 batches of x + matching mu/logvar rows)
and emits f32 partial sums; the host combines them in float64.

Layout (per core):
  mu/logvar: (128, 1024) SBUF, partition p = rows [16p, 16p+16)
             (contiguous 4KB per partition -> line-rate DMA).
  x:         (64, 2048) SBUF, d on partitions, column il = flat row il.
  "tile" n (n=0..15) = rows {16p + n}: column block n*64 of mu/logvar,
  columns n::16 of x (one (64,128)->(128,64) PE transpose each).
Engines: ACT: exp, PSUM->SBUF copies.  GPSIMD: mu*invv, x^2.
DVE: the three fused product+row-sum passes.  PE: transposes +
ones-matmuls (partition-direction sums, PSUM-accumulated).
"""

import sys

sys.path.insert(0, "/opt/trn_rl_repo")

import numpy as np
from contextlib import ExitStack

import concourse.bass as bass
import concourse.bacc as bacc
import concourse.tile as tile
from concourse import mybir
from concourse.masks import make_identity
from concourse.bass_utils import run_bass_kernel_spmd

F32 = mybir.dt.float32
N_CORES = 8
B, D, H, W = 16, 64, 32, 32
HW = H * W                # 1024
N = B * HW                # 16384
NB = B // N_CORES         # 2 batches per core
ROWS = NB * HW            # 2048 rows per core
NT = ROWS // 128          # 16 row-tiles per core
HALF = NT // 2 * D        # 512 columns per half-slab


def build_nc() -> bass.Bass:
    nc = bacc.Bacc()
    xs = nc.dram_tensor("xs", [NB, D, HW], F32, kind="ExternalInput")
    mus = nc.dram_tensor("mus", [ROWS, D], F32, kind="ExternalInput")
    lvs = nc.dram_tensor("lvs", [ROWS, D], F32, kind="ExternalInput")
    # accs columns: [A_h0 A_h1 | B_h0 B_h1 | C_h0 C_h1]
    accs = nc.dram_tensor("accs", [128, 6], F32, kind="ExternalOutput")
    # vrow blocks of 512: [S_invv | S_muinvv | S_x | S_x2], each (8,64) folded
    vrow = nc.dram_tensor("vrow", [1, 4 * HALF], F32, kind="ExternalOutput")

    with ExitStack() as ctx:
        tc = ctx.enter_context(tile.TileContext(nc))
        consts = ctx.enter_context(tc.tile_pool(name="consts", bufs=1))
        big = ctx.enter_context(tc.tile_pool(name="big", bufs=1))
        work = ctx.enter_context(tc.tile_pool(name="work", bufs=3))
        accp = ctx.enter_context(tc.tile_pool(name="accp", bufs=1))
        ptp = ctx.enter_context(tc.tile_pool(name="ptp", bufs=2, space="PSUM"))
        pacc = ctx.enter_context(tc.tile_pool(name="pacc", bufs=1, space="PSUM"))

        ident = consts.tile([64, 64], F32)
        make_identity(nc, ident)
        # zerob/ones on the scalar engine: zerob is the bias AP of ACT
        # activations and ones is read by PE matmuls; keeping their
        # producers on ACT/PE-adjacent chains minimizes sync-wait splits.
        zerob = consts.tile([128, 1], F32)
        nc.scalar.memzero(zerob[:])
        ones = consts.tile([128, 1], F32)
        nc.scalar.add(ones[:], zerob[:], 1.0)

        # ---- input DMAs ----
        xball = big.tile([64, ROWS], F32)
        for bl in range(NB):
            nc.sync.dma_start(out=xball[:, bl * HW:(bl + 1) * HW], in_=xs[bl])

        muall = big.tile([128, NT * D], F32)
        lvall = big.tile([128, NT * D], F32)
        mus_r = mus[:, :].rearrange("(p k) d -> p (k d)", p=128)
        lvs_r = lvs[:, :].rearrange("(p k) d -> p (k d)", p=128)
        for h in range(2):
            sl = slice(h * HALF, (h + 1) * HALF)
            nc.sync.dma_start(out=muall[:, sl], in_=mus_r[:, sl])
            nc.sync.dma_start(out=lvall[:, sl], in_=lvs_r[:, sl])

        # ---- persistent accumulators ----
        acc = accp.tile([128, 6], F32)
        psum_v = [
            pacc.tile([1, HALF], F32, tag=f"pv{k}", name=f"psum_v{k}")
            for k in range(4)
        ]

        invv = big.tile([128, NT * D], F32)
        mu_invv = big.tile([128, NT * D], F32)
        xt = big.tile([128, NT * D], F32)
        x2t = big.tile([128, NT * D], F32)

        # PE matmuls only support a single sync-wait in codegen.  The
        # warm-up transpose absorbs the wait on the gpsimd-built identity;
        # the two dummies absorb the two x-DMA queue sems (each real
        # transpose reads columns from both batches).
        xst = xball[:, :].rearrange("p (h s) -> p s h", s=16)
        warm_ps = ptp.tile([128, HALF], F32, tag="px")
        nc.tensor.transpose(warm_ps[0:64, 0:64], ident[:], ident[:])
        for bl in range(NB):
            nc.tensor.transpose(
                warm_ps[0:64, 0:64], xball[:, bl * HW:bl * HW + 64], ident[:]
            )

        for h in range(2):
            sl = slice(h * HALF, (h + 1) * HALF)
            lv_h = lvall[:, sl]
            mu_h = muall[:, sl]
            invv_h = invv[:, sl]
            mu_invv_h = mu_invv[:, sl]
            xt_h = xt[:, sl]
            x2t_h = x2t[:, sl]

            nc.scalar.activation(
                out=invv_h, in_=lv_h, func=mybir.ActivationFunctionType.Exp,
                bias=zerob[:], scale=-1.0,
            )
            nc.gpsimd.tensor_mul(mu_invv_h, mu_h, invv_h)

            scr = work.tile([128, HALF], F32, tag="scr")
            nc.vector.scalar_tensor_tensor(
                out=scr[:], in0=mu_h, scalar=1.0, in1=mu_invv_h,
                op0=mybir.AluOpType.mult, op1=mybir.AluOpType.mult,
                accum_out=acc[:, 4 + h:5 + h],
            )

            # transposes for this half: tiles n = 8h .. 8h+7
            px = warm_ps if h == 0 else ptp.tile([128, HALF], F32, tag="px")
            for k in range(8):
                n = 8 * h + k
                nc.tensor.transpose(
                    px[:, k * D:(k + 1) * D], xst[:, n, :], ident[:]
                )
            nc.scalar.copy(out=xt_h, in_=px[:])
            nc.gpsimd.tensor_mul(x2t_h, xt_h, xt_h)

            scr = work.tile([128, HALF], F32, tag="scr")
            nc.vector.scalar_tensor_tensor(
                out=scr[:], in0=x2t_h, scalar=1.0, in1=invv_h,
                op0=mybir.AluOpType.mult, op1=mybir.AluOpType.mult,
                accum_out=acc[:, 0 + h:1 + h],
            )
            scr = work.tile([128, HALF], F32, tag="scr")
            nc.vector.scalar_tensor_tensor(
                out=scr[:], in0=xt_h, scalar=1.0, in1=mu_invv_h,
                op0=mybir.AluOpType.mult, op1=mybir.AluOpType.mult,
                accum_out=acc[:, 2 + h:3 + h],
            )

            start, stop = (h == 0), (h == 1)
            for k, src in enumerate((invv_h, mu_invv_h, xt_h, x2t_h)):
                nc.tensor.matmul(
                    psum_v[k][:], ones[:], src, start=start, stop=stop
                )

        # ---- finalize ----
        vtile = accp.tile([1, 4 * HALF], F32)
        for k in range(4):
            nc.scalar.copy(
                out=vtile[0:1, k * HALF:(k + 1) * HALF], in_=psum_v[k][:]
            )
        nc.sync.dma_start(out=accs[:, :], in_=acc[:])
        nc.sync.dma_start(out=vrow[:, :], in_=vtile[:])
    return nc


_NC = None


def _get_nc():
    global _NC
    if _NC is None:
        _NC = build_nc()
        # bacc passes legalize multi-sync-wait instructions for TRN2 codegen
        _NC.compile()
    return _NC


def make_in_maps(x, mu, logvar):
    x = np.ascontiguousarray(np.asarray(x, dtype=np.float32))
    mu = np.asarray(mu, dtype=np.float32)
    lv = np.asarray(logvar, dtype=np.float32)
    in_maps = []
    for c in range(N_CORES):
        in_maps.append({
            "xs": x[c * NB:(c + 1) * NB].reshape(NB, D, HW),
            "mus": np.ascontiguousarray(mu[c * ROWS:(c + 1) * ROWS]),
            "lvs": np.ascontiguousarray(lv[c * ROWS:(c + 1) * ROWS]),
        })
    return in_maps


def combine(results) -> np.ndarray:
    A = 0.0
    Bs = 0.0
    C = 0.0
    V = np.zeros((4, D), dtype=np.float64)
    for r in results:
        a = np.asarray(r["accs"], dtype=np.float64)
        A += a[:, 0:2].sum()
        Bs += a[:, 2:4].sum()
        C += a[:, 4:6].sum()
        v = np.asarray(r["vrow"], dtype=np.float64).reshape(4, NT // 2, D)
        V += v.sum(axis=1)
    S_invv, S_muinvv, S_x, S_x2 = V
    mean_pos = -0.5 / N * (A - 2.0 * Bs + C)
    mean_D = (S_invv @ S_x2 - 2.0 * S_muinvv @ S_x + N * C) / float(N) ** 2
    loss = mean_pos + 0.5 * mean_D
    return np.array(loss, dtype=np.float32)


def kernel(x, mu, logvar, **_kwargs):
    nc = _get_nc()
    in_maps = make_in_maps(x, mu, logvar)
    res = run_bass_kernel_spmd(nc, in_maps, list(range(N_CORES)))
    return combine(res.results)


# revision 23
# speedup vs baseline: 1.2999x; 1.1110x over previous
"""CLUB loss kernel for Trainium2 (8 NeuronCores, SPMD row-sharded).

Math: the reference returns mean_i(pos_i - neg_i), a scalar.  Both the
pos and neg terms collapse into sums that never materialize the NxN
distance matrix:

  mean_pos = -0.5/N * (A - 2B + C)
      A = sum_{i,d} x[i,d]^2 * invv[i,d]
      B = sum_{i,d} x[i,d] * mu[i,d] * invv[i,d]
      C = sum_{i,d} mu[i,d]^2 * invv[i,d]
  mean_neg = -0.5 * (S_invv . S_x2 - 2 * S_muinvv . S_x + N*C) / N^2
      S_invv = sum_i invv[i,:]     S_muinvv = sum_i mu[i,:]*invv[i,:]
      S_x    = sum_j x[j,:]        S_x2     = sum_j x[j,:]^2
  loss = mean_pos - mean_neg

Each core handles 2048 rows (2 batches of x + matching mu/logvar rows)
and emits f32 partial sums; the host combines them in float64.

Layout: everything lives in the d-major layout (128, 1024): partition
q = (sub-slab b, dim d), free axis = row index within the sub-slab.
x arrives in this layout naturally (x[b] is (d, h*w) row-major); mu and
logvar are pre-transposed on the host as part of the shard layout.
With d on partitions every needed reduction is a free-axis row-sum, so
each quantity is one fused elementwise+accumulate instruction - no
on-chip transposes, no PSUM, no TensorEngine work at all (~20 compute
instructions per core).
"""

import sys

sys.path.insert(0, "/opt/trn_rl_repo")

import numpy as np
from contextlib import ExitStack

import concourse.bass as bass
import concourse.bacc as bacc
import concourse.tile as tile
from concourse import mybir
from concourse.bass_utils import run_bass_kernel_spmd

F32 = mybir.dt.float32
N_CORES = 8
B, D, H, W = 16, 64, 32, 32
HW = H * W                # 1024
N = B * HW                # 16384
NB = B // N_CORES         # 2 sub-slabs (batches) per core
ROWS = NB * HW            # 2048 rows per core
COLS = HW                 # free size of the (128, 1024) layout
# accum column map: quantity q, chunk c -> column q*NCH + c
QUANT = ["A", "B", "C", "Sx", "Sx2", "Sinvv", "Smuinvv"]
NCH = 2                   # accumulation chunks (bounds f32 chain length)
CW = COLS // NCH          # 512 columns per chunk


def build_nc() -> bass.Bass:
    nc = bacc.Bacc()
    xn = nc.dram_tensor("xn", [128, COLS], F32, kind="ExternalInput")
    mut = nc.dram_tensor("mut", [128, COLS], F32, kind="ExternalInput")
    lvt = nc.dram_tensor("lvt", [128, COLS], F32, kind="ExternalInput")
    accs = nc.dram_tensor("accs", [128, len(QUANT) * NCH], F32,
                          kind="ExternalOutput")

    with ExitStack() as ctx:
        tc = ctx.enter_context(tile.TileContext(nc))
        big = ctx.enter_context(tc.tile_pool(name="big", bufs=1))
        jp = ctx.enter_context(tc.tile_pool(name="jp", bufs=2))
        accp = ctx.enter_context(tc.tile_pool(name="accp", bufs=1))

        zerob = big.tile([128, 1], F32)
        nc.scalar.memzero(zerob[:])

        xb = big.tile([128, COLS], F32)
        mu = big.tile([128, COLS], F32)
        lv = big.tile([128, COLS], F32)
        for h in range(NCH):
            sl = slice(h * CW, (h + 1) * CW)
            nc.sync.dma_start(out=lv[:, sl], in_=lvt[:, sl])
            nc.sync.dma_start(out=mu[:, sl], in_=mut[:, sl])
            nc.sync.dma_start(out=xb[:, sl], in_=xn[:, sl])

        invv = big.tile([128, COLS], F32)
        muinvv = big.tile([128, COLS], F32)
        x2 = big.tile([128, COLS], F32)
        acc = accp.tile([128, len(QUANT) * NCH], F32)

        def col(q, c):
            return acc[:, QUANT.index(q) * NCH + c:QUANT.index(q) * NCH + c + 1]

        M = mybir.AluOpType.mult

        for h in range(NCH):
            sl = slice(h * CW, (h + 1) * CW)
            # ACT: invv = exp(-lv), accum -> S_invv ; x2 = x^2, accum -> S_x2
            nc.scalar.activation(
                out=invv[:, sl], in_=lv[:, sl],
                func=mybir.ActivationFunctionType.Exp,
                bias=zerob[:], scale=-1.0, accum_out=col("Sinvv", h),
            )
            nc.scalar.activation(
                out=x2[:, sl], in_=xb[:, sl],
                func=mybir.ActivationFunctionType.Square,
                bias=zerob[:], scale=1.0, accum_out=col("Sx2", h),
            )
            # GPSIMD: muinvv = mu * invv
            nc.gpsimd.tensor_mul(muinvv[:, sl], mu[:, sl], invv[:, sl])
            # ACT: S_muinvv row-sum (Copy with accumulate)
            ja = jp.tile([128, CW], F32, tag="ja")
            nc.scalar.activation(
                out=ja[:], in_=muinvv[:, sl],
                func=mybir.ActivationFunctionType.Copy,
                bias=0.0, scale=1.0, accum_out=col("Smuinvv", h),
            )
            # DVE: fused product + row-sum for A, B, C and S_x
            for q, in0, in1 in (
                ("A", x2, invv), ("B", xb, muinvv), ("C", mu, muinvv),
            ):
                jd = jp.tile([128, CW], F32, tag="jd")
                nc.vector.scalar_tensor_tensor(
                    out=jd[:], in0=in0[:, sl], scalar=1.0, in1=in1[:, sl],
                    op0=M, op1=M, accum_out=col(q, h),
                )
            jd = jp.tile([128, CW], F32, tag="jd")
            nc.vector.tensor_scalar(
                out=jd[:], in0=xb[:, sl], scalar1=1.0, scalar2=0.0,
                op0=M, op1=mybir.AluOpType.add, accum_out=col("Sx", h),
            )

        nc.sync.dma_start(out=accs[:, :], in_=acc[:])
    return nc


_NC = None


def _get_nc():
    global _NC
    if _NC is None:
        _NC = build_nc()
        # bacc passes legalize multi-sync-wait instructions for TRN2 codegen
        _NC.compile()
    return _NC


def make_in_maps(x, mu, logvar):
    x = np.ascontiguousarray(np.asarray(x, dtype=np.float32))
    mu = np.asarray(mu, dtype=np.float32)
    lv = np.asarray(logvar, dtype=np.float32)
    in_maps = []
    for c in range(N_CORES):
        r0 = c * ROWS
        mu_t = np.concatenate(
            [mu[r0 + b * HW:r0 + (b + 1) * HW].T for b in range(NB)], axis=0
        )
        lv_t = np.concatenate(
            [lv[r0 + b * HW:r0 + (b + 1) * HW].T for b in range(NB)], axis=0
        )
        in_maps.append({
            "xn": x[c * NB:(c + 1) * NB].reshape(128, COLS),
            "mut": np.ascontiguousarray(mu_t),
            "lvt": np.ascontiguousarray(lv_t),
        })
    return in_maps


def combine(results) -> np.ndarray:
    nq = len(QUANT)
    tot = np.zeros((nq, 128), dtype=np.float64)
    for r in results:
        a = np.asarray(r["accs"], dtype=np.float64)  # (128, nq*NCH)
        for q in range(nq):
            tot[q] += a[:, q * NCH:(q + 1) * NCH].sum(axis=1)
    scal = {q: tot[i].sum() for i, q in enumerate(QUANT[:3])}
    vec = {q: tot[i].reshape(NB, D).sum(axis=0)
           for i, q in enumerate(QUANT) if i >= 3}
    A, Bs, C = scal["A"], scal["B"], scal["C"]
    mean_pos = -0.5 / N * (A - 2.0 * Bs + C)
    mean_D = (vec["Sinvv"] @ vec["Sx2"] - 2.0 * vec["Smuinvv"] @ vec["Sx"]
              + N * C) / float(N) ** 2
    loss = mean_pos + 0.5 * mean_D
    return np.array(loss, dtype=np.float32)


def kernel(x, mu, logvar, **_kwargs):
    nc = _get_nc()
    in_maps = make_in_maps(x, mu, logvar)
    res = run_bass_kernel_spmd(nc, in_maps, list(range(N_CORES)))
    return combine(res.results)


# revision 24
# speedup vs baseline: 1.3534x; 1.0411x over previous
"""CLUB loss kernel for Trainium2 (8 NeuronCores, SPMD row-sharded).

Math: the reference returns mean_i(pos_i - neg_i), a scalar.  Both the
pos and neg terms collapse into sums that never materialize the NxN
distance matrix:

  mean_pos = -0.5/N * (A - 2B + C)
      A = sum_{i,d} x[i,d]^2 * invv[i,d]
      B = sum_{i,d} x[i,d] * mu[i,d] * invv[i,d]
      C = sum_{i,d} mu[i,d]^2 * invv[i,d]
  mean_neg = -0.5 * (S_invv . S_x2 - 2 * S_muinvv . S_x + N*C) / N^2
      S_invv = sum_i invv[i,:]     S_muinvv = sum_i mu[i,:]*invv[i,:]
      S_x    = sum_j x[j,:]        S_x2     = sum_j x[j,:]^2
  loss = mean_pos - mean_neg

Each core handles 2048 rows (2 batches of x + matching mu/logvar rows)
and emits f32 partial sums; the host combines them in float64.

Layout: everything lives in the d-major layout (128, 1024): partition
q = (sub-slab b, dim d), free axis = row index within the sub-slab.
x arrives in this layout naturally (x[b] is (d, h*w) row-major); mu and
logvar are pre-transposed on the host as part of the shard layout.
With d on partitions every needed reduction is a free-axis row-sum, so
each quantity is one fused elementwise+accumulate instruction - no
on-chip transposes, no PSUM, no TensorEngine work at all (~20 compute
instructions per core).
"""

import sys

sys.path.insert(0, "/opt/trn_rl_repo")

import numpy as np
from contextlib import ExitStack

import concourse.bass as bass
import concourse.bacc as bacc
import concourse.tile as tile
from concourse import mybir
from concourse.bass_utils import run_bass_kernel_spmd

F32 = mybir.dt.float32
N_CORES = 8
B, D, H, W = 16, 64, 32, 32
HW = H * W                # 1024
N = B * HW                # 16384
NB = B // N_CORES         # 2 sub-slabs (batches) per core
ROWS = NB * HW            # 2048 rows per core
COLS = HW                 # free size of the (128, 1024) layout
# accum column map: quantity q, chunk c -> column q*NCH + c
QUANT = ["A", "B", "C", "Sx", "Sx2", "Sinvv", "Smuinvv"]
NCH = 2                   # accumulation chunks (bounds f32 chain length)
CW = COLS // NCH          # 512 columns per chunk


def build_nc() -> bass.Bass:
    nc = bacc.Bacc()
    xn = nc.dram_tensor("xn", [128, COLS], F32, kind="ExternalInput")
    mut = nc.dram_tensor("mut", [128, COLS], F32, kind="ExternalInput")
    lvt = nc.dram_tensor("lvt", [128, COLS], F32, kind="ExternalInput")
    accs = nc.dram_tensor("accs", [128, len(QUANT) * NCH], F32,
                          kind="ExternalOutput")

    with ExitStack() as ctx:
        tc = ctx.enter_context(tile.TileContext(nc))
        big = ctx.enter_context(tc.tile_pool(name="big", bufs=1))
        jp = ctx.enter_context(tc.tile_pool(name="jp", bufs=2))
        accp = ctx.enter_context(tc.tile_pool(name="accp", bufs=1))

        zerob = big.tile([128, 1], F32)
        nc.scalar.memzero(zerob[:])

        xb = big.tile([128, COLS], F32)
        mu = big.tile([128, COLS], F32)
        lv = big.tile([128, COLS], F32)
        # Split DMA issue across both HWDGE engines (SP + ACT) so the six
        # descriptor generations don't serialize, ordered by when compute
        # needs each chunk (lv gates the exp chain).
        sl0 = slice(0, CW)
        sl1 = slice(CW, COLS)
        nc.sync.dma_start(out=lv[:, sl0], in_=lvt[:, sl0])
        nc.scalar.dma_start(out=lv[:, sl1], in_=lvt[:, sl1])
        nc.sync.dma_start(out=xb[:, sl0], in_=xn[:, sl0])
        nc.scalar.dma_start(out=xb[:, sl1], in_=xn[:, sl1])
        nc.sync.dma_start(out=mu[:, sl0], in_=mut[:, sl0])
        nc.scalar.dma_start(out=mu[:, sl1], in_=mut[:, sl1])

        invv = big.tile([128, COLS], F32)
        muinvv = big.tile([128, COLS], F32)
        x2 = big.tile([128, COLS], F32)
        acc = accp.tile([128, len(QUANT) * NCH], F32)

        def col(q, c):
            return acc[:, QUANT.index(q) * NCH + c:QUANT.index(q) * NCH + c + 1]

        M = mybir.AluOpType.mult
        sls = [slice(h * CW, (h + 1) * CW) for h in range(NCH)]

        def act(q, h, out, in_, func, scale=1.0):
            nc.scalar.activation(
                out=out, in_=in_, func=func, bias=zerob[:], scale=scale,
                accum_out=col(q, h),
            )

        def stt(q, h, in0, in1):
            jd = jp.tile([128, CW], F32, tag="jd", name=f"jd_{q}{h}")
            nc.vector.scalar_tensor_tensor(
                out=jd[:], in0=in0[:, sls[h]], scalar=1.0, in1=in1[:, sls[h]],
                op0=M, op1=M, accum_out=col(q, h),
            )

        EXP = mybir.ActivationFunctionType.Exp
        SQ = mybir.ActivationFunctionType.Square

        # Emission order = engine program order; DMA-gated ops first on each
        # engine, cross-engine-gated ops (muinvv consumers) last.
        act("Sinvv", 0, invv[:, sls[0]], lv[:, sls[0]], EXP, scale=-1.0)
        act("Sx2", 0, x2[:, sls[0]], xb[:, sls[0]], SQ)
        jd = jp.tile([128, CW], F32, tag="jd", name="jd_sx0")
        nc.vector.tensor_scalar(
            out=jd[:], in0=xb[:, sls[0]], scalar1=1.0, scalar2=0.0,
            op0=M, op1=mybir.AluOpType.add, accum_out=col("Sx", 0),
        )
        nc.gpsimd.tensor_mul(muinvv[:, sls[0]], mu[:, sls[0]], invv[:, sls[0]])
        act("Sinvv", 1, invv[:, sls[1]], lv[:, sls[1]], EXP, scale=-1.0)
        act("Sx2", 1, x2[:, sls[1]], xb[:, sls[1]], SQ)
        stt("A", 0, x2, invv)
        jd = jp.tile([128, CW], F32, tag="jd", name="jd_sx1")
        nc.vector.tensor_scalar(
            out=jd[:], in0=xb[:, sls[1]], scalar1=1.0, scalar2=0.0,
            op0=M, op1=mybir.AluOpType.add, accum_out=col("Sx", 1),
        )
        nc.gpsimd.tensor_mul(muinvv[:, sls[1]], mu[:, sls[1]], invv[:, sls[1]])
        stt("B", 0, xb, muinvv)
        stt("C", 0, mu, muinvv)
        ja = jp.tile([128, CW], F32, tag="ja", name="ja_0")
        nc.scalar.activation(
            out=ja[:], in_=muinvv[:, sls[0]],
            func=mybir.ActivationFunctionType.Copy,
            bias=0.0, scale=1.0, accum_out=col("Smuinvv", 0),
        )
        stt("A", 1, x2, invv)
        stt("B", 1, xb, muinvv)
        stt("C", 1, mu, muinvv)
        ja = jp.tile([128, CW], F32, tag="ja", name="ja_1")
        nc.scalar.activation(
            out=ja[:], in_=muinvv[:, sls[1]],
            func=mybir.ActivationFunctionType.Copy,
            bias=0.0, scale=1.0, accum_out=col("Smuinvv", 1),
        )

        nc.sync.dma_start(out=accs[:, :], in_=acc[:])
    return nc


_NC = None


def _get_nc():
    global _NC
    if _NC is None:
        _NC = build_nc()
        # bacc passes legalize multi-sync-wait instructions for TRN2 codegen
        _NC.compile()
    return _NC


def make_in_maps(x, mu, logvar):
    x = np.ascontiguousarray(np.asarray(x, dtype=np.float32))
    mu = np.asarray(mu, dtype=np.float32)
    lv = np.asarray(logvar, dtype=np.float32)
    in_maps = []
    for c in range(N_CORES):
        r0 = c * ROWS
        mu_t = np.concatenate(
            [mu[r0 + b * HW:r0 + (b + 1) * HW].T for b in range(NB)], axis=0
        )
        lv_t = np.concatenate(
            [lv[r0 + b * HW:r0 + (b + 1) * HW].T for b in range(NB)], axis=0
        )
        in_maps.append({
            "xn": x[c * NB:(c + 1) * NB].reshape(128, COLS),
            "mut": np.ascontiguousarray(mu_t),
            "lvt": np.ascontiguousarray(lv_t),
        })
    return in_maps


def combine(results) -> np.ndarray:
    nq = len(QUANT)
    tot = np.zeros((nq, 128), dtype=np.float64)
    for r in results:
        a = np.asarray(r["accs"], dtype=np.float64)  # (128, nq*NCH)
        for q in range(nq):
            tot[q] += a[:, q * NCH:(q + 1) * NCH].sum(axis=1)
    scal = {q: tot[i].sum() for i, q in enumerate(QUANT[:3])}
    vec = {q: tot[i].reshape(NB, D).sum(axis=0)
           for i, q in enumerate(QUANT) if i >= 3}
    A, Bs, C = scal["A"], scal["B"], scal["C"]
    mean_pos = -0.5 / N * (A - 2.0 * Bs + C)
    mean_D = (vec["Sinvv"] @ vec["Sx2"] - 2.0 * vec["Smuinvv"] @ vec["Sx"]
              + N * C) / float(N) ** 2
    loss = mean_pos + 0.5 * mean_D
    return np.array(loss, dtype=np.float32)


def kernel(x, mu, logvar, **_kwargs):
    nc = _get_nc()
    in_maps = make_in_maps(x, mu, logvar)
    res = run_bass_kernel_spmd(nc, in_maps, list(range(N_CORES)))
    return combine(res.results)
